# revision 19
# baseline (speedup 1.0000x reference)
"""Trainium2 Bass kernel for nn_DataEmbedding_ALLPE_Weighted.

Sharding: sequence-parallel over L (4096 tokens / 8 cores = 512 tokens per
core, all 16 batches). All parameters replicated; per-core slices of the PE
tables (learned_pe, tape_pos, R0 constants). The conv's circular-padding
halo (1 token each side) is precomputed on host and injected as a tiny
input, so there are no collectives.

Per-core pipeline:
  A. rolling stats via prefix sums / log-shift max-min chains in a
     [128 = 4 batch x 32 ch, 23+512] folded layout; comb assembled in
     [256 cin, 514] (channels-on-partitions) per batch via SBUF regroup
     DMAs; halo cols 0/513 come from the host tensor.
  B. pe_lin = tape_pos @ M2.T + c1  (4 token tiles, SBUF-resident)
  C. R0 = w2*LN_l + host-folded (w1*pf_ln + bias consts)  (4 tiles, SBUF)
  D. software-pipelined main loop over 64 (batch, token-tile) pairs:
       conv (6 bf16 MMs + rank-1 bias) -> LN-c (bn_stats) -> transpose ->
       mixer@tproj matmul + pe_lin -> LN-t -> out = PE(diag(w0 g_c)@hcT
       + R0) + DVE fused (ht * w3 g_t); final add on Pool.
"""

import numpy as np

import concourse.bass as bass
import concourse.mybir as mybir
import concourse.tile as tile
from concourse import bacc
from concourse.bass_utils import run_bass_kernel_spmd

F32 = mybir.dt.float32
BF16 = mybir.dt.bfloat16
AL = mybir.AluOpType
AF = mybir.ActivationFunctionType

B, L, C, D = 16, 4096, 32, 512
W = 24
LAGS = [3, 5, 7]
EPS = 1e-5
NCORES = 8
TPC = L // NCORES          # tokens per core: 512
NTL = TPC // 128           # token tiles per core: 4
PADL = W - 1               # 23
SEGP = TPC + PADL          # 535
NGA = 4                    # phase-A tile groups (4 batches each)
NTAP = 3
CW = TPC + 2               # comb columns per (batch, half): 514


def _build_bass():
    nc = bacc.Bacc("TRN2", target_bir_lowering=False, debug=False,
                   num_devices=NCORES)

    def din(name, shape, dt=F32):
        return nc.dram_tensor(name, shape, dt, kind="ExternalInput").ap()

    t_xpad = din("xpad", [NGA, 128, SEGP])
    t_halo = din("halo", [2, 128, 2 * B], BF16)
    t_wr = din("wr", [2 * NTAP, 128, 512], BF16)     # idx = tap*2 + half
    t_convb = din("convb", [1, 512], BF16)
    t_ftg = din("ftg", [4, 128, 512], BF16)
    t_m2t = din("m2t", [4, 128, 512], BF16)
    t_c1 = din("c1", [1, 512], BF16)
    t_tapet = din("tapet", [4, 128, TPC], BF16)
    t_pelpf = din("pelpf", [NTL, 128, 1024])
    t_g0 = din("g0bc", [128, 512], BF16)
    t_g3 = din("g3bc", [128, 512], BF16)
    t_gl = din("glbc", [128, 512])
    t_ident = din("ident", [128, 128], BF16)
    t_ones1 = din("ones1", [1, 128], BF16)
    t_out = nc.dram_tensor("out", [B, TPC, D], F32, kind="ExternalOutput").ap()

    with tile.TileContext(nc) as tc:
        _body(tc, nc, t_xpad, t_halo, t_wr, t_convb, t_ftg, t_m2t, t_c1,
              t_tapet, t_pelpf, t_g0, t_g3, t_gl, t_ident, t_ones1, t_out)
    nc.compile()
    return nc


def _body(tc, nc, t_xpad, t_halo, t_wr, t_convb, t_ftg, t_m2t, t_c1,
          t_tapet, t_pelpf, t_g0, t_g3, t_gl, t_ident, t_ones1, t_out):
    v = nc.vector
    gp = nc.gpsimd
    sc = nc.scalar
    pe = nc.tensor
    sy = nc.sync

    cpool = tc.alloc_tile_pool(name="consts", bufs=1)
    wr_s = cpool.tile([128, 6 * 512], BF16)
    sy.dma_start(wr_s[:].rearrange("p (k n) -> p k n", k=6),
                 t_wr.rearrange("k p n -> p k n"))
    ftg_s = cpool.tile([128, 4 * 512], BF16)
    sy.dma_start(ftg_s[:].rearrange("p (k n) -> p k n", k=4),
                 t_ftg.rearrange("k p n -> p k n"))
    m2t_s = cpool.tile([128, 4 * 512], BF16)
    sy.dma_start(m2t_s[:].rearrange("p (k n) -> p k n", k=4),
                 t_m2t.rearrange("k p n -> p k n"))
    g0_s = cpool.tile([128, 512], BF16)
    sy.dma_start(g0_s[:], t_g0)
    g3_s = cpool.tile([128, 512], BF16)
    sy.dma_start(g3_s[:], t_g3)
    gl_s = cpool.tile([128, 512], F32)
    sy.dma_start(gl_s[:], t_gl)
    ident_s = cpool.tile([128, 128], BF16)
    sy.dma_start(ident_s[:], t_ident)
    ones1_s = cpool.tile([1, 128], BF16)
    sy.dma_start(ones1_s[:], t_ones1)
    convb_s = cpool.tile([1, 512], BF16)
    sy.dma_start(convb_s[:], t_convb)
    c1_s = cpool.tile([1, 512], BF16)
    sy.dma_start(c1_s[:], t_c1)
    zscan_s = cpool.tile([128, SEGP], F32)
    gp.memset(zscan_s[:], 0.0)
    eps_s = cpool.tile([128, 1], F32)
    gp.memset(eps_s[:], EPS)

    pelin_sb = cpool.tile([128, NTL * 512], BF16)
    r0_sb = cpool.tile([128, NTL * 512], BF16)
    comb_sb = cpool.tile([128, 2 * B * CW], BF16)

    def comb_ap(b, half):
        off = (2 * b + half) * CW
        return comb_sb[:, off:off + CW]

    psum = tc.alloc_tile_pool(name="ps", bufs=2, space="PSUM")
    psph = tc.alloc_tile_pool(name="psph", bufs=3, space="PSUM")
    pa = tc.alloc_tile_pool(name="pa", bufs=1)
    pm = tc.alloc_tile_pool(name="pm", bufs=4)
    st = tc.alloc_tile_pool(name="st", bufs=6)
    pst = tc.alloc_tile_pool(name="pst", bufs=2)

    # halo cols 0 / 513 for every (batch, half), one strided DMA each
    comb3 = comb_sb[:].rearrange("p (g c) -> p g c", c=CW)
    sy.dma_start(comb3[:, :, 0:1], t_halo[0])
    sy.dma_start(comb3[:, :, CW - 1:CW], t_halo[1])

    # ---------------- Phase B: pe_lin = tape @ M2.T + c1 ----------------
    tgs = []
    for j in range(4):
        tg = pst.tile([128, TPC], BF16, tag=f"tapet{j}")
        sy.dma_start(tg[:], t_tapet[j])
        tgs.append(tg)
    for i in range(NTL):
        ps = psum.tile([128, 512], F32, tag="ppt")
        for j in range(4):
            pe.matmul(ps[:], tgs[j][:, 128 * i:128 * (i + 1)],
                      m2t_s[:, 512 * j:512 * (j + 1)],
                      start=(j == 0), stop=False)
        pe.matmul(ps[:], ones1_s[:], c1_s[:], start=False, stop=True)
        sc.copy(pelin_sb[:, 512 * i:512 * (i + 1)], ps[:])

    # ---------------- Phase C: R0 ----------------
    for i in range(NTL):
        plpf = pst.tile([128, 1024], F32, tag="plpf")
        sy.dma_start(plpf[:], t_pelpf[i])
        pl = plpf[:, 0:512]
        pf = plpf[:, 512:1024]
        alpha = st.tile([128, 1], F32, tag="al")
        beta = st.tile([128, 1], F32, tag="be")
        _ln_bn(nc, st, pl, alpha, beta, eps_s)
        hl = pm.tile([128, 512], BF16, tag="hl")
        sc.activation(hl[:], pl, AF.Identity, bias=beta[:], scale=alpha[:])
        t1 = pm.tile([128, 512], F32, tag="vA")
        v.tensor_tensor(t1[:], hl[:], gl_s[:], op=AL.mult)
        v.tensor_tensor(r0_sb[:, 512 * i:512 * (i + 1)], t1[:], pf, op=AL.add)

    # ---------------- Phase A: stats -> comb (per 4-batch group) --------
    def phase_a(g):
        xp = pa.tile([128, SEGP], F32, tag="xp")
        sy.dma_start(xp[:], t_xpad[g])
        xsq = pa.tile([128, SEGP], F32, tag="xsq")
        sc.square(xsq[:], xp[:])
        cs = pa.tile([128, SEGP + 1], F32, tag="cs")
        gp.memset(cs[:, 0:1], 0.0)
        v.tensor_tensor_scan(cs[:, 1:SEGP + 1], zscan_s[:], xp[:], 0.0,
                             op0=AL.add, op1=AL.add)
        cs2 = pa.tile([128, SEGP + 1], F32, tag="cs2")
        gp.memset(cs2[:, 0:1], 0.0)
        v.tensor_tensor_scan(cs2[:, 1:SEGP + 1], zscan_s[:], xsq[:], 0.0,
                             op0=AL.add, op1=AL.add)
        wsum = pa.tile([128, TPC], BF16, tag="wsum")
        v.tensor_sub(wsum[:], cs[:, W:W + TPC], cs[:, 0:TPC])
        ssqw = pa.tile([128, TPC], F32, tag="ssqw")
        v.tensor_sub(ssqw[:], cs2[:, W:W + TPC], cs2[:, 0:TPC])
        wsq = pa.tile([128, TPC], F32, tag="wsq")
        sc.square(wsq[:], wsum[:])
        var23 = pa.tile([128, TPC], F32, tag="var23")
        v.scalar_tensor_tensor(var23[:], wsq[:], -1.0 / W, ssqw[:],
                               op0=AL.mult, op1=AL.add)
        v.tensor_scalar_max(var23[:], var23[:], 0.0)
        stdt = pa.tile([128, TPC], BF16, tag="stdt")
        sc.activation(stdt[:], var23[:], AF.Sqrt, scale=1.0 / (W - 1))
        # max chain on Pool, min chain on DVE
        outs = {}
        for name, op, eng in (("mx", AL.max, v), ("mn", AL.min, v)):
            cht = "ch" + name
            m2 = pa.tile([128, SEGP - 1], F32, tag=cht + "2")
            eng.tensor_tensor(m2[:], xp[:, 0:SEGP - 1], xp[:, 1:SEGP], op=op)
            m4 = pa.tile([128, SEGP - 3], F32, tag=cht + "4")
            eng.tensor_tensor(m4[:], m2[:, 0:SEGP - 3], m2[:, 2:SEGP - 1],
                              op=op)
            m8 = pa.tile([128, SEGP - 7], F32, tag=cht + "8")
            eng.tensor_tensor(m8[:], m4[:, 0:SEGP - 7], m4[:, 4:SEGP - 3],
                              op=op)
            m16 = pa.tile([128, SEGP - 15], F32, tag=cht + "16")
            eng.tensor_tensor(m16[:], m8[:, 0:SEGP - 15], m8[:, 8:SEGP - 7],
                              op=op)
            mo = pa.tile([128, TPC], BF16, tag=name + "o")
            eng.tensor_tensor(mo[:], m16[:, 8:TPC + 8], m8[:, 0:TPC], op=op)
            outs[name] = mo
        lags = []
        for l in LAGS:
            lg = pa.tile([128, TPC], BF16, tag=f"lag{l}")
            v.tensor_sub(lg[:], xp[:, PADL:SEGP], xp[:, PADL - l:SEGP - l])
            lags.append(lg)
        xbf = pa.tile([128, TPC], BF16, tag="xbf")
        sc.copy(xbf[:], xp[:, PADL:SEGP])

        feats0 = [xbf, wsum, outs["mx"], outs["mn"]]
        feats1 = [stdt] + lags
        for bb in range(4):
            b = 4 * g + bb
            for half, feats in ((0, feats0), (1, feats1)):
                dst = comb_ap(b, half)
                for r, ft in enumerate(feats):
                    sy.dma_start(dst[32 * r:32 * (r + 1), 1:1 + TPC],
                                 ft[32 * bb:32 * (bb + 1), :])

    phase_a(0)

    # ---------------- Phase D: software-pipelined main loop ----------------
    NTOT = B * NTL
    state = {}

    def s0_conv(k):
        b, i = divmod(k, NTL)
        ph = psph.tile([128, 512], F32, tag="ph")
        kk = 0
        for tap in range(NTAP):
            for half in (0, 1):
                lhsT = comb_ap(b, half)[:, 128 * i + tap:128 * i + tap + 128]
                pe.matmul(ph[:], lhsT,
                          wr_s[:, 512 * (tap * 2 + half):
                               512 * (tap * 2 + half) + 512],
                          start=(kk == 0), stop=False)
                kk += 1
        pe.matmul(ph[:], ones1_s[:], convb_s[:], start=False, stop=True)
        state[k] = {"ph": ph}

    def s1_lnc(k):
        stt = state[k]
        ph = stt["ph"]
        al_c = st.tile([128, 1], F32, tag="al")
        be_c = st.tile([128, 1], F32, tag="be")
        _ln_bn(nc, st, ph[:], al_c, be_c, eps_s)
        hc = pm.tile([128, 512], BF16, tag="hc")
        sc.activation(hc[:], ph[:], AF.Identity, bias=be_c[:], scale=al_c[:])
        stt["hc"] = hc

    def s15_tp(k):
        stt = state[k]
        hc = stt["hc"]
        pT = psum.tile([128, 512], BF16, tag="pT")
        for j in range(4):
            pe.transpose(pT[:, 128 * j:128 * (j + 1)],
                         hc[:, 128 * j:128 * (j + 1)], ident_s[:])
        hcT = pm.tile([128, 512], BF16, tag="hcT")
        sc.copy(hcT[:], pT[:])
        po = pm.tile([128, 512], BF16, tag="pos")
        v.tensor_tensor(po[:], hc[:], g0_s[:], op=AL.mult)
        stt.update(hcT=hcT, po=po)

    def s2_mm(k):
        b, i = divmod(k, NTL)
        stt = state[k]
        hcT = stt["hcT"]
        ppt = psum.tile([128, 512], F32, tag="ppt")
        for j in range(4):
            pe.matmul(ppt[:], hcT[:, 128 * j:128 * (j + 1)],
                      ftg_s[:, 512 * j:512 * (j + 1)],
                      start=(j == 0), stop=False)
        pe.matmul(ppt[:], ident_s[:], pelin_sb[:, 512 * i:512 * (i + 1)],
                  start=False, stop=True)
        al_t = st.tile([128, 1], F32, tag="alt")
        be_t = st.tile([128, 1], F32, tag="bet")
        _ln_bn(nc, st, ppt[:], al_t, be_t, eps_s, tagsfx="t")
        stt.update(ppt=ppt, al_t=al_t, be_t=be_t)

    def s25_ht(k):
        stt = state[k]
        ht = pm.tile([128, 512], BF16, tag="ht")
        sc.activation(ht[:], stt["ppt"][:], AF.Identity, bias=stt["be_t"][:],
                      scale=stt["al_t"][:])
        stt["ht"] = ht

    def s3_out(k):
        b, i = divmod(k, NTL)
        stt = state.pop(k)
        tg = pm.tile([128, 512], BF16, tag="tg3")
        v.tensor_tensor(tg[:], stt["ht"][:], g3_s[:], op=AL.mult)
        t2 = pm.tile([128, 512], BF16, tag="t2")
        v.tensor_tensor(t2[:], tg[:], stt["po"][:], op=AL.add)
        osb = pm.tile([128, 512], BF16, tag="osb")
        v.tensor_tensor(osb[:], t2[:], r0_sb[:, 512 * i:512 * (i + 1)],
                        op=AL.add)
        gp.dma_start(t_out[b, 128 * i:128 * (i + 1), :], osb[:])

    for r in range(NTOT + 4):
        if r == 2:
            phase_a(1)
        elif r == 16:
            phase_a(2)
        elif r == 32:
            phase_a(3)
        if r >= 4:
            s25_ht(r - 4)
        if 3 <= r <= NTOT + 2:
            s2_mm(r - 3)
        if 2 <= r <= NTOT + 1:
            s15_tp(r - 2)
        if 1 <= r <= NTOT:
            s1_lnc(r - 1)
        if r < NTOT:
            s0_conv(r)
        if r >= 4:
            s3_out(r - 4)

    for p in (pst, st, pm, pa, psph, psum, cpool):
        p.release()


def _ln_bn(nc, st, src, alpha, beta, eps_s, tagsfx=""):
    """One-pass LN stats: bn_stats/bn_aggr -> alpha=1/sqrt(var+eps),
    beta=-mean*alpha."""
    v = nc.vector
    sc = nc.scalar
    stats = st.tile([128, 6], F32, tag="bns" + tagsfx)
    v.bn_stats(stats[:], src)
    mv = st.tile([128, 2], F32, tag="mv" + tagsfx)
    v.bn_aggr(mv[:], stats[:])
    sc.activation(alpha[:], mv[:, 1:2], AF.Abs_reciprocal_sqrt,
                  bias=eps_s[:])
    v.scalar_tensor_tensor(beta[:], mv[:, 0:1], -1.0, alpha[:],
                           op0=AL.mult, op1=AL.mult)


_NC_CACHE = None


def _get_nc():
    global _NC_CACHE
    if _NC_CACHE is None:
        _NC_CACHE = _build_bass()
    return _NC_CACHE


def _host_prep(inputs):
    import ml_dtypes

    f32 = np.float32

    def to_bf16(a):
        return np.asarray(a, f32).astype(ml_dtypes.bfloat16)

    x = np.asarray(inputs["x"], f32)
    conv_w = np.asarray(inputs["conv_w"], f32)
    conv_b = np.asarray(inputs["conv_b"], f32)
    learned_pe = np.asarray(inputs["learned_pe"], f32)
    tape_pos = np.asarray(inputs["tape_pos"], f32)
    tproj_w = np.asarray(inputs["tproj_w"], f32)
    tproj_b = np.asarray(inputs["tproj_b"], f32)
    mixer_w = np.asarray(inputs["mixer_w"], f32)
    mixer_b = np.asarray(inputs["mixer_b"], f32)
    g_c, b_c = np.asarray(inputs["g_c"], f32), np.asarray(inputs["b_c"], f32)
    g_f, b_f = np.asarray(inputs["g_f"], f32), np.asarray(inputs["b_f"], f32)
    g_l, b_l = np.asarray(inputs["g_l"], f32), np.asarray(inputs["b_l"], f32)
    g_t, b_t = np.asarray(inputs["g_t"], f32), np.asarray(inputs["b_t"], f32)
    wp = np.asarray(inputs["weight_params"], f32)

    e = np.exp(wp - wp.max())
    w = (e / e.sum()).astype(f32)

    # conv weights: fold mean scale, rearrange to [tap*2+half, 128, 512]
    cw = conv_w.copy()
    cw[:, C:2 * C, :] /= W
    wr = np.empty((6, 128, 512), f32)
    for tap in range(NTAP):
        for half in (0, 1):
            wr[tap * 2 + half] = cw[:, 128 * half:128 * (half + 1), tap].T

    M1 = mixer_w[:, :D]
    M2 = mixer_w[:, D:]
    F = M1 @ tproj_w
    F_g = F * g_c[None, :]
    c1 = F @ b_c + M1 @ tproj_b + mixer_b
    ftg = np.ascontiguousarray(F_g.T).reshape(4, 128, 512)
    m2t = np.ascontiguousarray(M2.T).reshape(4, 128, 512)

    # fixed sinusoidal PE table, normalized (pure constant)
    pos = np.arange(L, dtype=np.float64)
    div = np.exp(np.arange(0, D, 2, dtype=np.float64) * (-np.log(10000.0) / D))
    ang = pos[:, None] * div[None, :]
    tab = np.zeros((L, D), np.float64)
    tab[:, 0::2] = np.sin(ang)
    tab[:, 1::2] = np.cos(ang)
    tab = tab.astype(f32)
    mtab = tab.mean(-1, keepdims=True)
    vtab = ((tab - mtab) ** 2).mean(-1, keepdims=True)
    tabn = (tab - mtab) / np.sqrt(vtab + EPS)
    pfg = (w[1] * (g_f[None, :] * tabn + b_f[None, :])
           + (w[0] * b_c + w[3] * b_t + w[2] * b_l)[None, :]).astype(f32)

    # x padded on the left by PADL with edge replication: index t+PADL <-> x[t]
    xp_full = np.pad(x, ((0, 0), (PADL, 0), (0, 0)), mode="edge")

    def comb_feats(t):
        """comb feature columns for global token t: ([B,128], [B,128])."""
        win = xp_full[:, t:t + W, :]                       # [B, W, C]
        s = win.sum(1)
        mx = win.max(1)
        mn = win.min(1)
        m = win.mean(1)
        sd = np.sqrt(((win - m[:, None, :]) ** 2).sum(1) / (W - 1))
        xt = x[:, t, :]
        lgs = [x[:, t, :] - xp_full[:, t + PADL - l, :] for l in LAGS]
        half0 = np.concatenate([xt, s, mx, mn], axis=-1)   # [B, 128]
        half1 = np.concatenate([sd] + lgs, axis=-1)        # [B, 128]
        return half0, half1

    base = {
        "wr": to_bf16(wr),
        "convb": to_bf16(conv_b[None, :]),
        "ftg": to_bf16(ftg),
        "m2t": to_bf16(m2t),
        "c1": to_bf16(c1[None, :]),
        "g0bc": to_bf16(np.tile((w[0] * g_c)[None, :], (128, 1))),
        "g3bc": to_bf16(np.tile((w[3] * g_t)[None, :], (128, 1))),
        "glbc": np.tile((w[2] * g_l)[None, :], (128, 1)).astype(f32),
        "ident": to_bf16(np.eye(128, dtype=f32)),
        "ones1": to_bf16(np.ones((1, 128), f32)),
    }
    in_maps = []
    for c in range(NCORES):
        T0 = TPC * c
        T1 = T0 + TPC
        # xpad[g, 32*bb+ch, j] = x-token (T0-23+j) of batch 4g+bb
        xpc = xp_full[:, T0:T0 + SEGP, :]                  # [B, 535, C]
        xpad = np.ascontiguousarray(
            xpc.transpose(0, 2, 1)).reshape(NGA, 128, SEGP)
        # halo comb cols: token T0-1 (left) and T1 (right), circular
        tl = (T0 - 1) % L
        tr = T1 % L
        halo = np.zeros((2, 128, 2 * B), f32)
        for side, t in ((0, tl), (1, tr)):
            h0, h1 = comb_feats(t)
            halo[side, :, 0::2] = h0.T
            halo[side, :, 1::2] = h1.T
        pelpf = np.concatenate(
            [learned_pe[0, T0:T1].reshape(NTL, 128, 512),
             pfg[T0:T1].reshape(NTL, 128, 512)], axis=2).copy()
        tapet = np.ascontiguousarray(
            tape_pos[T0:T1].T).reshape(4, 128, TPC)
        m = dict(base)
        m["xpad"] = xpad
        m["halo"] = to_bf16(halo)
        m["pelpf"] = pelpf
        m["tapet"] = to_bf16(tapet)
        in_maps.append(m)
    return in_maps


def kernel(**inputs):
    in_maps = _host_prep(inputs)
    nc = _get_nc()
    res = run_bass_kernel_spmd(nc, in_maps, core_ids=list(range(NCORES)))
    out = np.concatenate([r["out"] for r in res.results], axis=1)
    return out.astype(np.float32)


# revision 20
# speedup vs baseline: 1.0836x; 1.0836x over previous
"""Trainium2 Bass kernel for nn_DataEmbedding_ALLPE_Weighted.

Sharding: sequence-parallel over L (4096 tokens / 8 cores = 512 tokens per
core, all 16 batches). All parameters replicated; per-core slices of the PE
tables (learned_pe, tape_pos, R0 constants). The conv's circular-padding
halo (1 token each side) is precomputed on host and injected as a tiny
input, so there are no collectives.

Per-core pipeline:
  A. rolling stats via prefix sums / log-shift max-min chains in a
     [128 = 4 batch x 32 ch, 23+512] folded layout; comb assembled in
     [256 cin, 514] (channels-on-partitions) per batch via SBUF regroup
     DMAs; halo cols 0/513 come from the host tensor.
  B. pe_lin = tape_pos @ M2.T + c1  (4 token tiles, SBUF-resident)
  C. R0 = w2*LN_l + host-folded (w1*pf_ln + bias consts)  (4 tiles, SBUF)
  D. software-pipelined main loop over 64 (batch, token-tile) pairs:
       conv (6 bf16 MMs + rank-1 bias) -> LN-c (bn_stats) -> transpose ->
       mixer@tproj matmul + pe_lin -> LN-t -> out = PE(diag(w0 g_c)@hcT
       + R0) + DVE fused (ht * w3 g_t); final add on Pool.
"""

import numpy as np

import concourse.bass as bass
import concourse.mybir as mybir
import concourse.tile as tile
from concourse import bacc
from concourse.bass_utils import run_bass_kernel_spmd

F32 = mybir.dt.float32
BF16 = mybir.dt.bfloat16
AL = mybir.AluOpType
AF = mybir.ActivationFunctionType

B, L, C, D = 16, 4096, 32, 512
W = 24
LAGS = [3, 5, 7]
EPS = 1e-5
NCORES = 8
TPC = L // NCORES          # tokens per core: 512
NTL = TPC // 128           # token tiles per core: 4
PADL = W - 1               # 23
SEGP = TPC + PADL          # 535
NGA = 4                    # phase-A tile groups (4 batches each)
NTAP = 3
CW = TPC + 2               # comb columns per (batch, half): 514


def _build_bass():
    nc = bacc.Bacc("TRN2", target_bir_lowering=False, debug=False,
                   num_devices=NCORES)

    def din(name, shape, dt=F32):
        return nc.dram_tensor(name, shape, dt, kind="ExternalInput").ap()

    t_xpad = din("xpad", [NGA, 128, SEGP])
    t_halo = din("halo", [2, 128, 2 * B], BF16)
    t_wr = din("wr", [2 * NTAP, 128, 512], BF16)     # idx = tap*2 + half
    t_convb = din("convb", [1, 512], BF16)
    t_ftg = din("ftg", [4, 128, 512], BF16)
    t_m2t = din("m2t", [4, 128, 512], BF16)
    t_c1 = din("c1", [1, 512], BF16)
    t_tapet = din("tapet", [4, 128, TPC], BF16)
    t_pelpf = din("pelpf", [NTL, 128, 1024])
    t_g0 = din("g0bc", [128, 512], BF16)
    t_g3 = din("g3bc", [128, 512], BF16)
    t_gl = din("glbc", [128, 512])
    t_ident = din("ident", [128, 128], BF16)
    t_ones1 = din("ones1", [1, 128], BF16)
    t_out = nc.dram_tensor("out", [B, TPC, D], F32, kind="ExternalOutput").ap()

    with tile.TileContext(nc) as tc:
        _body(tc, nc, t_xpad, t_halo, t_wr, t_convb, t_ftg, t_m2t, t_c1,
              t_tapet, t_pelpf, t_g0, t_g3, t_gl, t_ident, t_ones1, t_out)
    nc.compile()
    return nc


def _body(tc, nc, t_xpad, t_halo, t_wr, t_convb, t_ftg, t_m2t, t_c1,
          t_tapet, t_pelpf, t_g0, t_g3, t_gl, t_ident, t_ones1, t_out):
    v = nc.vector
    gp = nc.gpsimd
    sc = nc.scalar
    pe = nc.tensor
    sy = nc.sync

    cpool = tc.alloc_tile_pool(name="consts", bufs=1)
    wr_s = cpool.tile([128, 6 * 512], BF16)
    sy.dma_start(wr_s[:].rearrange("p (k n) -> p k n", k=6),
                 t_wr.rearrange("k p n -> p k n"))
    ftg_s = cpool.tile([128, 4 * 512], BF16)
    sy.dma_start(ftg_s[:].rearrange("p (k n) -> p k n", k=4),
                 t_ftg.rearrange("k p n -> p k n"))
    m2t_s = cpool.tile([128, 4 * 512], BF16)
    sy.dma_start(m2t_s[:].rearrange("p (k n) -> p k n", k=4),
                 t_m2t.rearrange("k p n -> p k n"))
    g0_s = cpool.tile([128, 512], BF16)
    sy.dma_start(g0_s[:], t_g0)
    g3_s = cpool.tile([128, 512], BF16)
    sy.dma_start(g3_s[:], t_g3)
    gl_s = cpool.tile([128, 512], F32)
    sy.dma_start(gl_s[:], t_gl)
    ident_s = cpool.tile([128, 128], BF16)
    sy.dma_start(ident_s[:], t_ident)
    ones1_s = cpool.tile([1, 128], BF16)
    sy.dma_start(ones1_s[:], t_ones1)
    convb_s = cpool.tile([1, 512], BF16)
    sy.dma_start(convb_s[:], t_convb)
    c1_s = cpool.tile([1, 512], BF16)
    sy.dma_start(c1_s[:], t_c1)
    zscan_s = cpool.tile([128, SEGP], F32)
    gp.memset(zscan_s[:], 0.0)
    eps_s = cpool.tile([128, 1], F32)
    gp.memset(eps_s[:], EPS)

    pelin_sb = cpool.tile([128, NTL * 512], BF16)
    r0_sb = cpool.tile([128, NTL * 512], BF16)
    comb_sb = cpool.tile([128, 2 * B * CW], BF16)

    def comb_ap(b, half):
        off = (2 * b + half) * CW
        return comb_sb[:, off:off + CW]

    psum = tc.alloc_tile_pool(name="ps", bufs=2, space="PSUM")
    psph = tc.alloc_tile_pool(name="psph", bufs=3, space="PSUM")
    pa = tc.alloc_tile_pool(name="pa", bufs=1)
    pm = tc.alloc_tile_pool(name="pm", bufs=4)
    st = tc.alloc_tile_pool(name="st", bufs=6)
    pst = tc.alloc_tile_pool(name="pst", bufs=2)

    # halo cols 0 / 513 for every (batch, half), one strided DMA each
    comb3 = comb_sb[:].rearrange("p (g c) -> p g c", c=CW)
    sy.dma_start(comb3[:, :, 0:1], t_halo[0])
    sy.dma_start(comb3[:, :, CW - 1:CW], t_halo[1])

    # ---------------- Phase B: pe_lin = tape @ M2.T + c1 ----------------
    tgs = []
    for j in range(4):
        tg = pst.tile([128, TPC], BF16, tag=f"tapet{j}")
        sy.dma_start(tg[:], t_tapet[j])
        tgs.append(tg)
    for i in range(NTL):
        ps = psum.tile([128, 512], F32, tag="ppt")
        for j in range(4):
            pe.matmul(ps[:], tgs[j][:, 128 * i:128 * (i + 1)],
                      m2t_s[:, 512 * j:512 * (j + 1)],
                      start=(j == 0), stop=False)
        pe.matmul(ps[:], ones1_s[:], c1_s[:], start=False, stop=True)
        sc.copy(pelin_sb[:, 512 * i:512 * (i + 1)], ps[:])

    # ---------------- Phase C: R0 ----------------
    for i in range(NTL):
        plpf = pst.tile([128, 1024], F32, tag="plpf")
        sy.dma_start(plpf[:], t_pelpf[i])
        pl = plpf[:, 0:512]
        pf = plpf[:, 512:1024]
        alpha = st.tile([128, 1], F32, tag="al")
        beta = st.tile([128, 1], F32, tag="be")
        _ln_bn(nc, st, pl, alpha, beta, eps_s)
        hl = pm.tile([128, 512], BF16, tag="hl")
        sc.activation(hl[:], pl, AF.Identity, bias=beta[:], scale=alpha[:])
        t1 = pm.tile([128, 512], F32, tag="vA")
        v.tensor_tensor(t1[:], hl[:], gl_s[:], op=AL.mult)
        v.tensor_tensor(r0_sb[:, 512 * i:512 * (i + 1)], t1[:], pf, op=AL.add)

    # ---------------- Phase A: stats -> comb (per 4-batch group) --------
    def phase_a(g):
        xp = pa.tile([128, SEGP], F32, tag="xp")
        sy.dma_start(xp[:], t_xpad[g])
        xsq = pa.tile([128, SEGP], F32, tag="xsq")
        sc.square(xsq[:], xp[:])
        cs = pa.tile([128, SEGP + 1], F32, tag="cs")
        gp.memset(cs[:, 0:1], 0.0)
        v.tensor_tensor_scan(cs[:, 1:SEGP + 1], zscan_s[:], xp[:], 0.0,
                             op0=AL.add, op1=AL.add)
        cs2 = pa.tile([128, SEGP + 1], F32, tag="cs2")
        gp.memset(cs2[:, 0:1], 0.0)
        v.tensor_tensor_scan(cs2[:, 1:SEGP + 1], zscan_s[:], xsq[:], 0.0,
                             op0=AL.add, op1=AL.add)
        wsum = pa.tile([128, TPC], BF16, tag="wsum")
        v.tensor_sub(wsum[:], cs[:, W:W + TPC], cs[:, 0:TPC])
        ssqw = pa.tile([128, TPC], F32, tag="ssqw")
        v.tensor_sub(ssqw[:], cs2[:, W:W + TPC], cs2[:, 0:TPC])
        wsq = pa.tile([128, TPC], F32, tag="wsq")
        sc.square(wsq[:], wsum[:])
        var23 = pa.tile([128, TPC], F32, tag="var23")
        v.scalar_tensor_tensor(var23[:], wsq[:], -1.0 / W, ssqw[:],
                               op0=AL.mult, op1=AL.add)
        v.tensor_scalar_max(var23[:], var23[:], 0.0)
        stdt = pa.tile([128, TPC], BF16, tag="stdt")
        sc.activation(stdt[:], var23[:], AF.Sqrt, scale=1.0 / (W - 1))
        # max chain on Pool, min chain on DVE
        outs = {}
        for name, op, eng in (("mx", AL.max, v), ("mn", AL.min, v)):
            cht = "ch" + name
            m2 = pa.tile([128, SEGP - 1], F32, tag=cht + "2")
            eng.tensor_tensor(m2[:], xp[:, 0:SEGP - 1], xp[:, 1:SEGP], op=op)
            m4 = pa.tile([128, SEGP - 3], F32, tag=cht + "4")
            eng.tensor_tensor(m4[:], m2[:, 0:SEGP - 3], m2[:, 2:SEGP - 1],
                              op=op)
            m8 = pa.tile([128, SEGP - 7], F32, tag=cht + "8")
            eng.tensor_tensor(m8[:], m4[:, 0:SEGP - 7], m4[:, 4:SEGP - 3],
                              op=op)
            m16 = pa.tile([128, SEGP - 15], F32, tag=cht + "16")
            eng.tensor_tensor(m16[:], m8[:, 0:SEGP - 15], m8[:, 8:SEGP - 7],
                              op=op)
            mo = pa.tile([128, TPC], BF16, tag=name + "o")
            eng.tensor_tensor(mo[:], m16[:, 8:TPC + 8], m8[:, 0:TPC], op=op)
            outs[name] = mo
        lags = []
        for l in LAGS:
            lg = pa.tile([128, TPC], BF16, tag=f"lag{l}")
            v.tensor_sub(lg[:], xp[:, PADL:SEGP], xp[:, PADL - l:SEGP - l])
            lags.append(lg)
        xbf = pa.tile([128, TPC], BF16, tag="xbf")
        sc.copy(xbf[:], xp[:, PADL:SEGP])

        feats0 = [xbf, wsum, outs["mx"], outs["mn"]]
        feats1 = [stdt] + lags
        for bb in range(4):
            b = 4 * g + bb
            for half, feats in ((0, feats0), (1, feats1)):
                dst = comb_ap(b, half)
                for r, ft in enumerate(feats):
                    sy.dma_start(dst[32 * r:32 * (r + 1), 1:1 + TPC],
                                 ft[32 * bb:32 * (bb + 1), :])

    phase_a(0)

    # ---------------- Phase D: software-pipelined main loop ----------------
    NTOT = B * NTL
    state = {}

    def s0_conv(k):
        b, i = divmod(k, NTL)
        ph = psph.tile([128, 512], F32, tag="ph")
        kk = 0
        for tap in range(NTAP):
            for half in (0, 1):
                lhsT = comb_ap(b, half)[:, 128 * i + tap:128 * i + tap + 128]
                pe.matmul(ph[:], lhsT,
                          wr_s[:, 512 * (tap * 2 + half):
                               512 * (tap * 2 + half) + 512],
                          start=(kk == 0), stop=False)
                kk += 1
        pe.matmul(ph[:], ones1_s[:], convb_s[:], start=False, stop=True)
        state[k] = {"ph": ph}

    def s1_lnc(k):
        stt = state[k]
        ph = stt["ph"]
        al_c = st.tile([128, 1], F32, tag="al")
        be_c = st.tile([128, 1], F32, tag="be")
        _ln_bn(nc, st, ph[:], al_c, be_c, eps_s)
        hc = pm.tile([128, 512], BF16, tag="hc")
        sc.activation(hc[:], ph[:], AF.Identity, bias=be_c[:], scale=al_c[:])
        stt["hc"] = hc

    def s15_tp(k):
        stt = state[k]
        hc = stt["hc"]
        pT = psum.tile([128, 512], BF16, tag="pT")
        for j in range(4):
            pe.transpose(pT[:, 128 * j:128 * (j + 1)],
                         hc[:, 128 * j:128 * (j + 1)], ident_s[:])
        hcT = pm.tile([128, 512], BF16, tag="hcT")
        sc.copy(hcT[:], pT[:])
        po = pm.tile([128, 512], BF16, tag="pos")
        v.tensor_tensor(po[:], hc[:], g0_s[:], op=AL.mult)
        stt.update(hcT=hcT, po=po)

    def s2_mm(k):
        b, i = divmod(k, NTL)
        stt = state[k]
        hcT = stt["hcT"]
        ppt = psum.tile([128, 512], F32, tag="ppt")
        for j in range(4):
            pe.matmul(ppt[:], hcT[:, 128 * j:128 * (j + 1)],
                      ftg_s[:, 512 * j:512 * (j + 1)],
                      start=(j == 0), stop=False)
        pe.matmul(ppt[:], ident_s[:], pelin_sb[:, 512 * i:512 * (i + 1)],
                  start=False, stop=True)
        al_t = st.tile([128, 1], F32, tag="alt")
        be_t = st.tile([128, 1], F32, tag="bet")
        _ln_bn(nc, st, ppt[:], al_t, be_t, eps_s, tagsfx="t")
        stt.update(ppt=ppt, al_t=al_t, be_t=be_t)

    def s25_ht(k):
        stt = state[k]
        ht = pm.tile([128, 512], BF16, tag="ht")
        sc.activation(ht[:], stt["ppt"][:], AF.Identity, bias=stt["be_t"][:],
                      scale=stt["al_t"][:])
        stt["ht"] = ht

    def s3_out(k):
        b, i = divmod(k, NTL)
        stt = state.pop(k)
        tg = pm.tile([128, 512], BF16, tag="tg3")
        v.tensor_tensor(tg[:], stt["ht"][:], g3_s[:], op=AL.mult)
        t2 = pm.tile([128, 512], BF16, tag="t2")
        v.tensor_tensor(t2[:], tg[:], stt["po"][:], op=AL.add)
        osb = pm.tile([128, 512], BF16, tag="osb")
        v.tensor_tensor(osb[:], t2[:], r0_sb[:, 512 * i:512 * (i + 1)],
                        op=AL.add)
        gp.dma_start(t_out[b, 128 * i:128 * (i + 1), :], osb[:])

    for r in range(NTOT + 5):
        if r == 2:
            phase_a(1)
        elif r == 16:
            phase_a(2)
        elif r == 32:
            phase_a(3)
        if 4 <= r <= NTOT + 3:
            s25_ht(r - 4)
        if 3 <= r <= NTOT + 2:
            s2_mm(r - 3)
        if 2 <= r <= NTOT + 1:
            s15_tp(r - 2)
        if 1 <= r <= NTOT:
            s1_lnc(r - 1)
        if r < NTOT:
            s0_conv(r)
        if r >= 5:
            s3_out(r - 5)

    for p in (pst, st, pm, pa, psph, psum, cpool):
        p.release()


def _ln_bn(nc, st, src, alpha, beta, eps_s, tagsfx=""):
    """One-pass LN stats: bn_stats/bn_aggr -> alpha=1/sqrt(var+eps),
    beta=-mean*alpha."""
    v = nc.vector
    sc = nc.scalar
    stats = st.tile([128, 6], F32, tag="bns" + tagsfx)
    v.bn_stats(stats[:], src)
    mv = st.tile([128, 2], F32, tag="mv" + tagsfx)
    v.bn_aggr(mv[:], stats[:])
    sc.activation(alpha[:], mv[:, 1:2], AF.Abs_reciprocal_sqrt,
                  bias=eps_s[:])
    v.scalar_tensor_tensor(beta[:], mv[:, 0:1], -1.0, alpha[:],
                           op0=AL.mult, op1=AL.mult)


_NC_CACHE = None


def _get_nc():
    global _NC_CACHE
    if _NC_CACHE is None:
        _NC_CACHE = _build_bass()
    return _NC_CACHE


def _host_prep(inputs):
    import ml_dtypes

    f32 = np.float32

    def to_bf16(a):
        return np.asarray(a, f32).astype(ml_dtypes.bfloat16)

    x = np.asarray(inputs["x"], f32)
    conv_w = np.asarray(inputs["conv_w"], f32)
    conv_b = np.asarray(inputs["conv_b"], f32)
    learned_pe = np.asarray(inputs["learned_pe"], f32)
    tape_pos = np.asarray(inputs["tape_pos"], f32)
    tproj_w = np.asarray(inputs["tproj_w"], f32)
    tproj_b = np.asarray(inputs["tproj_b"], f32)
    mixer_w = np.asarray(inputs["mixer_w"], f32)
    mixer_b = np.asarray(inputs["mixer_b"], f32)
    g_c, b_c = np.asarray(inputs["g_c"], f32), np.asarray(inputs["b_c"], f32)
    g_f, b_f = np.asarray(inputs["g_f"], f32), np.asarray(inputs["b_f"], f32)
    g_l, b_l = np.asarray(inputs["g_l"], f32), np.asarray(inputs["b_l"], f32)
    g_t, b_t = np.asarray(inputs["g_t"], f32), np.asarray(inputs["b_t"], f32)
    wp = np.asarray(inputs["weight_params"], f32)

    e = np.exp(wp - wp.max())
    w = (e / e.sum()).astype(f32)

    # conv weights: fold mean scale, rearrange to [tap*2+half, 128, 512]
    cw = conv_w.copy()
    cw[:, C:2 * C, :] /= W
    wr = np.empty((6, 128, 512), f32)
    for tap in range(NTAP):
        for half in (0, 1):
            wr[tap * 2 + half] = cw[:, 128 * half:128 * (half + 1), tap].T

    M1 = mixer_w[:, :D]
    M2 = mixer_w[:, D:]
    F = M1 @ tproj_w
    F_g = F * g_c[None, :]
    c1 = F @ b_c + M1 @ tproj_b + mixer_b
    ftg = np.ascontiguousarray(F_g.T).reshape(4, 128, 512)
    m2t = np.ascontiguousarray(M2.T).reshape(4, 128, 512)

    # fixed sinusoidal PE table, normalized (pure constant)
    pos = np.arange(L, dtype=np.float64)
    div = np.exp(np.arange(0, D, 2, dtype=np.float64) * (-np.log(10000.0) / D))
    ang = pos[:, None] * div[None, :]
    tab = np.zeros((L, D), np.float64)
    tab[:, 0::2] = np.sin(ang)
    tab[:, 1::2] = np.cos(ang)
    tab = tab.astype(f32)
    mtab = tab.mean(-1, keepdims=True)
    vtab = ((tab - mtab) ** 2).mean(-1, keepdims=True)
    tabn = (tab - mtab) / np.sqrt(vtab + EPS)
    pfg = (w[1] * (g_f[None, :] * tabn + b_f[None, :])
           + (w[0] * b_c + w[3] * b_t + w[2] * b_l)[None, :]).astype(f32)

    # x padded on the left by PADL with edge replication: index t+PADL <-> x[t]
    xp_full = np.pad(x, ((0, 0), (PADL, 0), (0, 0)), mode="edge")

    def comb_feats(t):
        """comb feature columns for global token t: ([B,128], [B,128])."""
        win = xp_full[:, t:t + W, :]                       # [B, W, C]
        s = win.sum(1)
        mx = win.max(1)
        mn = win.min(1)
        m = win.mean(1)
        sd = np.sqrt(((win - m[:, None, :]) ** 2).sum(1) / (W - 1))
        xt = x[:, t, :]
        lgs = [x[:, t, :] - xp_full[:, t + PADL - l, :] for l in LAGS]
        half0 = np.concatenate([xt, s, mx, mn], axis=-1)   # [B, 128]
        half1 = np.concatenate([sd] + lgs, axis=-1)        # [B, 128]
        return half0, half1

    base = {
        "wr": to_bf16(wr),
        "convb": to_bf16(conv_b[None, :]),
        "ftg": to_bf16(ftg),
        "m2t": to_bf16(m2t),
        "c1": to_bf16(c1[None, :]),
        "g0bc": to_bf16(np.tile((w[0] * g_c)[None, :], (128, 1))),
        "g3bc": to_bf16(np.tile((w[3] * g_t)[None, :], (128, 1))),
        "glbc": np.tile((w[2] * g_l)[None, :], (128, 1)).astype(f32),
        "ident": to_bf16(np.eye(128, dtype=f32)),
        "ones1": to_bf16(np.ones((1, 128), f32)),
    }
    in_maps = []
    for c in range(NCORES):
        T0 = TPC * c
        T1 = T0 + TPC
        # xpad[g, 32*bb+ch, j] = x-token (T0-23+j) of batch 4g+bb
        xpc = xp_full[:, T0:T0 + SEGP, :]                  # [B, 535, C]
        xpad = np.ascontiguousarray(
            xpc.transpose(0, 2, 1)).reshape(NGA, 128, SEGP)
        # halo comb cols: token T0-1 (left) and T1 (right), circular
        tl = (T0 - 1) % L
        tr = T1 % L
        halo = np.zeros((2, 128, 2 * B), f32)
        for side, t in ((0, tl), (1, tr)):
            h0, h1 = comb_feats(t)
            halo[side, :, 0::2] = h0.T
            halo[side, :, 1::2] = h1.T
        pelpf = np.concatenate(
            [learned_pe[0, T0:T1].reshape(NTL, 128, 512),
             pfg[T0:T1].reshape(NTL, 128, 512)], axis=2).copy()
        tapet = np.ascontiguousarray(
            tape_pos[T0:T1].T).reshape(4, 128, TPC)
        m = dict(base)
        m["xpad"] = xpad
        m["halo"] = to_bf16(halo)
        m["pelpf"] = pelpf
        m["tapet"] = to_bf16(tapet)
        in_maps.append(m)
    return in_maps


def kernel(**inputs):
    in_maps = _host_prep(inputs)
    nc = _get_nc()
    res = run_bass_kernel_spmd(nc, in_maps, core_ids=list(range(NCORES)))
    out = np.concatenate([r["out"] for r in res.results], axis=1)
    return out.astype(np.float32)
